# revision 1
# baseline (speedup 1.0000x reference)
"""Contrastive loss (InfoNCE-style) on 8 Trainium2 NeuronCores.

Reference math (B=8192, D=128, temp=0.07):
    sim = (emb @ emb.T) / temp, diag masked to -1e9
    log_probs = log_softmax(sim, axis=1)
    row_mean_i = mean over positives (same label, j != i) of log_probs[i, :]
    loss = -sum(row_mean_i) / count(rows with >=1 positive)

Decomposition used here:
    log_probs[i, j] = sim[i, j] - lse_i,   lse_i = log(sum_{j!=i} exp(sim[i, j]))
    pos_sum_i  = q_i - pc_i * lse_i, where q_i = sum_{j pos} sim[i, j] (exact,
                 computed on host in f64 via class-summed embeddings) and
                 pc_i = (# rows with same label) - 1 (host, exact integer math)
    => the ONLY O(B^2) quantity is esum_i = sum_{j!=i} exp(sim[i, j]).

Device kernel (per core c, SPMD-uniform via column rotation):
    input embT_c = roll(emb.T, -1024*c, axis=1)  [128, 8192] f32
      -> core c's own 1024 rows are local columns 0..1023, and row-tile t's
         diagonal element sits at local column 128*t + p (p = partition).
    for each of 8 row-tiles [128 rows]:
      16 matmuls (fp32r, N=512) -> PSUM quarters [128, 2048]
      additive diag mask (-30000) on the [128,128] diagonal block (quarter 0)
      scalar-engine activation Exp(in * 1/temp) with accum_out -> per-quarter
      row sums; exp never leaves SBUF scratch.
    output esums [128, 8] (partition p, row-tile t).

Host: lse = log(esum); row_mean = q/pc - lse (where pc>0); reduce to scalar.
"""

import os

import numpy as np

import concourse.bass as bass
import concourse.mybir as mybir
import concourse.tile as tile
from concourse.tile import add_dep_helper
from concourse.bass_utils import run_bass_kernel_spmd

TEMP = 0.07
B = 8192
D = 128
NCORES = 8
RPC = B // NCORES        # 1024 rows per core
NT = RPC // 128          # 8 row-tiles of 128 rows per core
NCH = B // 512           # 16 column chunks of 512
MASK_RAW = -30000.0      # added in raw-dot space; exp((x+MASK)/temp) == 0.0

_CACHE = {}

# test.py introspection: last BassKernelResults from run_bass_kernel_spmd.
last_results = None


def _build_bass():
    f32 = mybir.dt.float32
    f32r = mybir.dt.float32r
    nc = bass.Bass("TRN2", target_bir_lowering=False, debug=False,
                   num_devices=NCORES)
    # blob0: ident (cols 0:128) | mask strip (128:1152) | emb cols 0:2048
    # (1152:3200). blobR: emb cols 2048:8192. One DMA each keeps the DMA-queue
    # semaphore count low enough for walrus's per-instruction sync limits.
    blob0 = nc.dram_tensor("blob0", [128, 3200], f32r, kind="ExternalInput")
    blob1 = nc.dram_tensor("blob1", [128, 6144], f32r, kind="ExternalInput")
    esums = nc.dram_tensor("esums", [128, NT], f32, kind="ExternalOutput")

    with tile.TileContext(nc) as tc:
        with (
            tc.tile_pool(name="big", bufs=1) as big,
            tc.tile_pool(name="psum", bufs=2, space="PSUM") as psum,
            tc.tile_pool(name="scratch", bufs=32) as scratch,
            tc.tile_pool(name="small", bufs=1) as small,
        ):
            blob0_s = big.tile([128, 3200], f32r)
            nc.sync.dma_start(out=blob0_s[:, :], in_=blob0.ap()[:, :])
            in_dma0 = nc.cur_bb.bb.instructions[-1]
            embR = big.tile([128, 6144], f32r)
            nc.sync.dma_start(out=embR[:, :], in_=blob1.ap()[:, :])
            in_dma1 = nc.cur_bb.bb.instructions[-1]
            # manual drains observing each input queue on the SP proc, so the
            # wait-limited kernel-tail drain doesn't need those semaphores
            for dep in (in_dma0, in_dma1):
                nc.sync.drain()
                add_dep_helper(nc.cur_bb.bb.instructions[-1], dep, sync=True,
                               reason="observe input DMA queue on SP")
            ident_s = blob0_s[:, 0:128]
            mstrip_s = blob0_s[:, 128:1152]
            emb0 = blob0_s[:, 1152:3200]

            esum_all = small.tile([128, NT * 4], f32)
            esums_s = small.tile([128, NT], f32)

            # prefetch dummies: a discarded LDWEIGHTS per input DMA, so the
            # PE observes every DMA semaphore here and real matmuls never
            # need to carry more than one sync wait (walrus limit); real
            # matmuls reload their own weights, so the garbage load is inert
            bf16 = mybir.dt.bfloat16
            nc.tensor.ldweights(blob0_s[:, 0:1].bitcast(bf16))
            nc.tensor.ldweights(embR[:, 0:1].bitcast(bf16))

            for t in range(NT):
                lhs = emb0[:, t * 128:(t + 1) * 128]
                kd, od = t // 4, (t % 4) * 128   # diag chunk within quarter 0
                for q in range(4):
                    qi = t * 4 + q
                    ps = psum.tile([128, 2048], f32, tag="ps")
                    carrier = None
                    if qi >= 2:
                        # discarded LDWEIGHTS reading the 2-quarters-ago accum
                        # slice: carries the ACT wait so the slot-reuse matmul
                        # below carries only its PE wait
                        nc.tensor.ldweights(
                            esum_all[:, qi - 2:qi - 1].bitcast(bf16))
                        carrier = nc.cur_bb.bb.instructions[-1]
                    for k in range(4):
                        n = 4 * q + k
                        is_diag = (q == 0 and k == kd)
                        nc.tensor.matmul(
                            ps[:, k * 512:(k + 1) * 512],
                            lhs,
                            (emb0[:, n * 512:(n + 1) * 512] if n < 4 else
                             embR[:, (n - 4) * 512:(n - 3) * 512]),
                            start=True, stop=not is_diag,
                        )
                        if carrier is not None:
                            add_dep_helper(nc.cur_bb.bb.instructions[-1],
                                           carrier, sync=False,
                                           reason="wait-carrier order")
                            carrier = None
                        if is_diag:
                            # accumulate -1e4 onto the diagonal entries:
                            # out[m, n] += sum_k I[k, m] * mstrip[k, n]
                            nc.tensor.matmul(
                                ps[:, k * 512:(k + 1) * 512],
                                ident_s,
                                mstrip_s[:, 512 - od:1024 - od],
                                start=False, stop=True,
                            )
                        last_mm = nc.cur_bb.bb.instructions[-1]
                    scr = scratch.tile([128, 2048], mybir.dt.bfloat16)
                    nc.scalar.activation(
                        scr[:, :], ps[:, :],
                        mybir.ActivationFunctionType.Exp,
                        scale=1.0 / TEMP,
                        accum_out=esum_all[:, t * 4 + q: t * 4 + q + 1],
                    )

            # final [128, 4] -> [128, 1] sums per row-tile on the scalar
            # engine (keeps the vector engine out of the program: fewer
            # semaphores on walrus's wait-limited kernel-tail drain)
            junk = small.tile([128, 4 * NT], f32)
            for t in range(NT):
                nc.scalar.activation(
                    junk[:, t * 4:(t + 1) * 4],
                    esum_all[:, t * 4:(t + 1) * 4],
                    mybir.ActivationFunctionType.Copy,
                    accum_out=esums_s[:, t:t + 1],
                )
            last_act = nc.cur_bb.bb.instructions[-1]
            # one manual drain per outstanding proc, each carrying a single
            # wait, so the auto-generated kernel-tail drain (which tolerates
            # almost no sync waits) has nothing left to wait for
            nc.sync.drain()
            add_dep_helper(nc.cur_bb.bb.instructions[-1], last_mm, sync=True,
                           reason="observe PE on SP")
            nc.sync.drain()
            add_dep_helper(nc.cur_bb.bb.instructions[-1], last_act, sync=True,
                           reason="observe ACT on SP")
            nc.sync.dma_start(out=esums.ap()[:, :], in_=esums_s[:, :])
            out_dma = nc.cur_bb.bb.instructions[-1]
            nc.sync.drain()
            add_dep_helper(nc.cur_bb.bb.instructions[-1], out_dma, sync=True,
                           reason="observe out DMA queue on SP")
    return nc


def _get_nc():
    if "nc" not in _CACHE:
        _CACHE["nc"] = _build_bass()
    return _CACHE["nc"]


def _host_inputs(emb):
    """Per-core in_maps: rotated emb.T plus the diagonal mask tile."""
    embT = np.ascontiguousarray(emb.T.astype(np.float32, copy=False))  # [128, B]
    qidx = np.arange(1024)[None, :]
    pidx = np.arange(128)[:, None]
    maskc = np.concatenate([
        np.eye(128, dtype=np.float32),
        np.where(qidx == pidx + 512, MASK_RAW, 0.0).astype(np.float32),
    ], axis=1)
    in_maps = []
    for c in range(NCORES):
        s = RPC * c
        rolled = np.concatenate([embT[:, s:], embT[:, :s]], axis=1)
        in_maps.append({
            "blob0": np.ascontiguousarray(
                np.concatenate([maskc, rolled[:, 0:2048]], axis=1)),
            "blob1": np.ascontiguousarray(rolled[:, 2048:]),
        })
    return in_maps


def kernel(embeddings, labels):
    global last_results
    emb = np.asarray(embeddings, dtype=np.float32)
    labels = np.asarray(labels).astype(np.int64)
    assert emb.shape == (B, D) and labels.shape == (B,)

    nc = _get_nc()
    in_maps = _host_inputs(emb)
    res = run_bass_kernel_spmd(nc, in_maps, core_ids=list(range(NCORES)))
    last_results = res

    # esums[p, t] on core c -> global row 1024*c + 128*t + p
    esum = np.concatenate(
        [np.asarray(res.results[c]["esums"]).T.reshape(-1) for c in range(NCORES)]
    ).astype(np.float64)

    emb64 = emb.astype(np.float64)
    nclass = int(labels.max()) + 1
    cnt = np.bincount(labels, minlength=nclass)
    pc = cnt[labels] - 1                      # positives per row (excl. self)
    G = np.zeros((nclass, D), dtype=np.float64)
    np.add.at(G, labels, emb64)
    # q_i = sum over positives j (same label, j != i) of sim[i, j]
    q = (np.einsum("ij,ij->i", emb64, G[labels])
         - np.einsum("ij,ij->i", emb64, emb64)) / TEMP

    lse = np.log(esum)
    has = pc > 0
    row_mean = np.where(has, q / np.maximum(pc, 1) - lse, 0.0)
    loss = -row_mean.sum() / max(int(has.sum()), 1)
    return np.float32(loss)



# revision 17
# speedup vs baseline: 3.5128x; 3.5128x over previous
"""Contrastive loss (InfoNCE-style) on 8 Trainium2 NeuronCores.

Reference math (B=8192, D=128, temp=0.07):
    sim = (emb @ emb.T) / temp, diag masked to -1e9
    log_probs = log_softmax(sim, axis=1)
    row_mean_i = mean over positives (same label, j != i) of log_probs[i, :]
    loss = -sum(row_mean_i) / count(rows with >=1 positive)

Decomposition used here:
    log_probs[i, j] = sim[i, j] - lse_i,   lse_i = log(sum_{j!=i} exp(sim[i, j]))
    pos_sum_i  = q_i - pc_i * lse_i, where q_i = sum_{j pos} sim[i, j] (exact,
                 computed on host in f64 via class-summed embeddings) and
                 pc_i = (# rows with same label) - 1 (host, exact integer math)
    => the ONLY O(B^2) quantity is esum_i = sum_{j!=i} exp(sim[i, j]).

Sharding (the big change vs the 1.0 s/call baseline): the wall-clock metric
is dominated by host->device transfer through the axon tunnel (~50 MB/s), so
instead of shipping a per-core rotated copy of the full [128, 8192] table
(8 x 4.8 MB = 38 MB), each core receives ONLY its own 512 KB shard
embT_c = emb[1024c:1024(c+1)].T and the full table is rebuilt ON DEVICE with
a DRAM AllGather over NeuronLink (4 MB total upload, ~10x less traffic).

Device kernel (per core, SPMD-uniform, no rotation needed):
    - DMA own shard [128, 1024] -> SBUF (lhs source)
    - DRAM bounce + AllGather -> agout [8*128, 1024]; 8 DMAs -> embT [128, 8192]
    - self-blocks: per row-tile t, matmul lhs_t^T lhs_t -> diag holds raw
      s_ii; affine_select keeps the diagonal (fill -30000), ACT Exp accum
      -> expd[:, t] = exp(s_ii/temp), bit-identical to the diag term inside
      the main sum (same PE/ACT datapath on same operand bits)
    - main: per tile t, 4 quarters x 4 matmuls [128,512] (fp32r) -> PSUM,
      ACT Exp(in/temp) with accum_out -> per-quarter row sums (f32 scratch)
    - output esums [128, 16]: cols 0:8 total exp-sums (incl. self term),
      cols 8:16 exp(diag). Host: esum_excl = total - expd in f64 (exact).

Host: lse = log(esum_excl); row_mean = q/pc - lse (where pc>0); reduce.
"""

import numpy as np

import concourse.bass as bass
import concourse.mybir as mybir
import concourse.tile as tile
from concourse.tile import add_dep_helper
from concourse.bass_utils import run_bass_kernel_spmd

TEMP = 0.07
B = 8192
D = 128
NCORES = 8
RPC = B // NCORES        # 1024 rows per core
NT = RPC // 128          # 8 row-tiles of 128 rows per core
MASK_RAW = -30000.0      # raw-dot space; exp(MASK/temp) == 0.0 in f32

_CACHE = {}

# test.py introspection: last BassKernelResults from run_bass_kernel_spmd.
last_results = None


def _build_bass():
    f32 = mybir.dt.float32
    f32r = mybir.dt.float32r
    bf16 = mybir.dt.bfloat16
    nc = bass.Bass("TRN2", target_bir_lowering=False, debug=False,
                   num_devices=NCORES)
    eshard = nc.dram_tensor("eshard", [128, RPC], f32r, kind="ExternalInput")
    esums = nc.dram_tensor("esums", [128, 2 * NT], f32, kind="ExternalOutput")

    with tile.TileContext(nc) as tc:
        with (
            tc.tile_pool(name="big", bufs=1) as big,
            tc.tile_pool(name="psum", bufs=2, space="PSUM") as psum,
            tc.tile_pool(name="scratch", bufs=32) as scratch,
            tc.tile_pool(name="small", bufs=1) as small,
            tc.tile_pool(name="dram", bufs=1, space="DRAM") as dram,
        ):
            shard_s = big.tile([128, RPC], f32r)
            nc.sync.dma_start(out=shard_s[:, :], in_=eshard.ap()[:, :])
            in_dma0 = nc.cur_bb.bb.instructions[-1]
            nc.sync.drain()
            add_dep_helper(nc.cur_bb.bb.instructions[-1], in_dma0, sync=True,
                           reason="observe input DMA queue on SP")

            # AllGather: input bounce (collectives can't touch I/O tensors),
            # gather to a Shared DRAM scratch, then 8 DMAs rebuild the full
            # [128, 8192] column table in SBUF in natural global order.
            agin = dram.tile([128, RPC], f32r)
            agout = dram.tile([NCORES, 128, RPC], f32r, addr_space="Shared")
            nc.gpsimd.dma_start(out=agin[:, :], in_=eshard.ap()[:, :])
            agin_dma = nc.cur_bb.bb.instructions[-1]
            nc.gpsimd.collective_compute(
                "AllGather", mybir.AluOpType.bypass,
                replica_groups=[list(range(NCORES))],
                ins=[agin.opt()], outs=[agout.opt()],
            )
            cc_inst = nc.cur_bb.bb.instructions[-1]
            embT = big.tile([128, B], f32r)
            # ONE multi-dim DMA for all 8 gathered pieces: walking the DRAM
            # side [c, p, j] -> [p, c, j] lands piece c at SBUF columns
            # [1024c, 1024(c+1)). A single DMA keeps every queue at one
            # entry (walrus allows only one sync wait per DMA entry) and
            # carries the collective wait for the whole gather.
            nc.sync.dma_start(
                out=embT[:, :].rearrange("p (c j) -> p c j", c=NCORES),
                in_=agout[:, :, :].transpose([1, 0, 2]),
            )
            gather_dmas = [nc.cur_bb.bb.instructions[-1]]

            esum_all = small.tile([128, NT * 4], f32)
            esums_s = small.tile([128, 2 * NT], f32)

            # prefetch dummy: a discarded LDWEIGHTS observing the shard DMA,
            # so real matmuls don't carry that queue wait (walrus limit)
            nc.tensor.ldweights(shard_s[:, 0:1].bitcast(bf16))

            # --- self blocks (only need the own shard; overlaps the gather)
            ps_self = psum.tile([128, 2048], f32, tag="ps")
            for t in range(NT):
                lhs = shard_s[:, t * 128:(t + 1) * 128]
                nc.tensor.matmul(ps_self[:, t * 128:(t + 1) * 128], lhs, lhs,
                                 start=True, stop=True)
            # prefetch dummy: a discarded LDWEIGHTS observing the gather DMA
            # on PE, placed after the self matmuls so those still overlap
            # the collective; main matmuls then never carry the gather-queue
            # wait and stay within walrus's one-sync-wait limit
            nc.tensor.ldweights(embT[:, B - 1:B].bitcast(bf16))
            sb_all = small.tile([128, NT * 128], f32)
            nc.scalar.activation(sb_all[:, :], ps_self[:, 0:NT * 128],
                                 mybir.ActivationFunctionType.Copy)
            sbm = small.tile([128, NT * 128], f32)
            nc.gpsimd.affine_select(
                sbm[:, :], sb_all[:, :], pattern=[[0, NT], [-1, 128]],
                compare_op=mybir.AluOpType.is_equal, fill=MASK_RAW,
                base=0, channel_multiplier=1,
            )
            asel_inst = nc.cur_bb.bb.instructions[-1]
            # bf16 like the main-path scratch: the diag's output rounding then
            # matches the main sum's diag term bit-for-bit and cancels exactly
            junkd = small.tile([128, NT * 128], mybir.dt.bfloat16)
            for t in range(NT):
                nc.scalar.activation(
                    junkd[:, t * 128:(t + 1) * 128],
                    sbm[:, t * 128:(t + 1) * 128],
                    mybir.ActivationFunctionType.Exp, scale=1.0 / TEMP,
                    accum_out=esums_s[:, NT + t:NT + t + 1],
                )

            # --- main loop: 8 row-tiles x 4 quarters x 4 matmuls of [128,512]
            for t in range(NT):
                lhs = shard_s[:, t * 128:(t + 1) * 128]
                for q in range(4):
                    qi = t * 4 + q
                    a = qi + 1            # psum alloc index (ps_self was 0)
                    ps = psum.tile([128, 2048], f32, tag="ps")
                    carrier = None
                    if a >= 2:
                        # discarded LDWEIGHTS reading the 2-allocations-ago
                        # ACT result: carries the psum-WAR ACT wait so the
                        # slot-reuse matmul below carries only its own wait
                        obs = (sb_all[:, 0:1] if a == 2
                               else esum_all[:, a - 3:a - 2])
                        nc.tensor.ldweights(obs.bitcast(bf16))
                        carrier = nc.cur_bb.bb.instructions[-1]
                    for k in range(4):
                        n = 4 * q + k
                        nc.tensor.matmul(
                            ps[:, k * 512:(k + 1) * 512],
                            lhs,
                            embT[:, n * 512:(n + 1) * 512],
                            start=True, stop=True,
                        )
                        if carrier is not None:
                            add_dep_helper(nc.cur_bb.bb.instructions[-1],
                                           carrier, sync=False,
                                           reason="wait-carrier order")
                            carrier = None
                        last_mm = nc.cur_bb.bb.instructions[-1]
                    scr = scratch.tile([128, 2048], mybir.dt.bfloat16)
                    nc.scalar.activation(
                        scr[:, :], ps[:, :],
                        mybir.ActivationFunctionType.Exp,
                        scale=1.0 / TEMP,
                        accum_out=esum_all[:, qi:qi + 1],
                    )

            # final [128, 4] -> [128, 1] sums per row-tile on the scalar
            # engine (keeps the vector engine out of the program)
            junk = small.tile([128, 4 * NT], f32)
            for t in range(NT):
                nc.scalar.activation(
                    junk[:, t * 4:(t + 1) * 4],
                    esum_all[:, t * 4:(t + 1) * 4],
                    mybir.ActivationFunctionType.Copy,
                    accum_out=esums_s[:, t:t + 1],
                )
            last_act = nc.cur_bb.bb.instructions[-1]
            # one manual drain per outstanding proc, each carrying a single
            # wait, so the auto-generated kernel-tail drain (which tolerates
            # almost no sync waits) has nothing left to wait for
            nc.sync.drain()
            add_dep_helper(nc.cur_bb.bb.instructions[-1], last_mm, sync=True,
                           reason="observe PE on SP")
            nc.sync.drain()
            add_dep_helper(nc.cur_bb.bb.instructions[-1], last_act, sync=True,
                           reason="observe ACT on SP")
            nc.sync.drain()
            add_dep_helper(nc.cur_bb.bb.instructions[-1], agin_dma, sync=True,
                           reason="observe gpsimd DMA queue on SP")
            nc.sync.drain()
            add_dep_helper(nc.cur_bb.bb.instructions[-1], cc_inst, sync=True,
                           reason="observe collective on SP")
            nc.sync.drain()
            add_dep_helper(nc.cur_bb.bb.instructions[-1], asel_inst, sync=True,
                           reason="observe gpsimd engine on SP")
            for g in gather_dmas:
                nc.sync.drain()
                add_dep_helper(nc.cur_bb.bb.instructions[-1], g, sync=True,
                               reason="observe gather DMA queue on SP")
            nc.sync.dma_start(out=esums.ap()[:, :], in_=esums_s[:, :])
            out_dma = nc.cur_bb.bb.instructions[-1]
            nc.sync.drain()
            add_dep_helper(nc.cur_bb.bb.instructions[-1], out_dma, sync=True,
                           reason="observe out DMA queue on SP")
    return nc


def _get_nc():
    if "nc" not in _CACHE:
        _CACHE["nc"] = _build_bass()
    return _CACHE["nc"]


def _host_inputs(emb):
    """Per-core in_maps: just the core's own [128, 1024] embT shard."""
    return [{"eshard": np.ascontiguousarray(emb[RPC * c:RPC * (c + 1)].T)}
            for c in range(NCORES)]


def kernel(embeddings, labels):
    global last_results
    emb = np.asarray(embeddings, dtype=np.float32)
    labels = np.asarray(labels).astype(np.int64)
    assert emb.shape == (B, D) and labels.shape == (B,)

    nc = _get_nc()
    in_maps = _host_inputs(emb)
    res = run_bass_kernel_spmd(nc, in_maps, core_ids=list(range(NCORES)))
    last_results = res

    # esums[p, 0:8] full exp-sum, [p, 8:16] exp(diag); local row = 128*t + p
    outs = [np.asarray(res.results[c]["esums"]) for c in range(NCORES)]
    tot = np.concatenate([o[:, :NT].T.reshape(-1) for o in outs]).astype(np.float64)
    expd = np.concatenate([o[:, NT:].T.reshape(-1) for o in outs]).astype(np.float64)
    esum = tot - expd

    emb64 = emb.astype(np.float64)
    nclass = int(labels.max()) + 1
    cnt = np.bincount(labels, minlength=nclass)
    pc = cnt[labels] - 1                      # positives per row (excl. self)
    G = np.zeros((nclass, D), dtype=np.float64)
    np.add.at(G, labels, emb64)
    # q_i = sum over positives j (same label, j != i) of sim[i, j]
    q = (np.einsum("ij,ij->i", emb64, G[labels])
         - np.einsum("ij,ij->i", emb64, emb64)) / TEMP

    lse = np.log(esum)
    has = pc > 0
    row_mean = np.where(has, q / np.maximum(pc, 1) - lse, 0.0)
    loss = -row_mean.sum() / max(int(has.sum()), 1)
    return np.float32(loss)
